# revision 7
# baseline (speedup 1.0000x reference)
"""DGI (Deep Graph Infomax) forward kernel for 8 TRN2 NeuronCores.

Problem (all shapes hardcoded):
  seq1, seq2: [1, 8192, 128] f32   node features
  adj:        [1, 8192, 8192] f32  dense adjacency
  cc_label:   [8, 1024] i32        community partition (arange layout)
  W: [128,128], b: [128], Wb: [128,128], bb: [] f32
  out:        [1, 16384] f32       = concat(ret1, ret2)

Math per GCN branch: h = relu(adj @ (seq @ W) + b). We reassociate to
(adj @ seq) @ W so the big contraction uses natural-layout seq tiles as
the stationary operand and a host-transposed adj block as the moving
operand; everything then lives in "transposed" space (features on
partitions), where the community mean is a free-axis reduction and the
bilinear scores are a 1-column matmul.

Sharding: core k owns nodes [1024k, 1024k+1024) == community k (cc_label
is arange). Each core reads its adjT column block (16 MB fp16), the full
seqs (4 MB fp16, replicated), computes its 1024 scores per branch. No
collectives.

Per-core device program (everything on the PE in fp16 so nothing runs at
the fp32 quarter rate; adj is pre-scaled by 256 on the host to sit in
fp16's normal range; the inverse scale is folded into W on the host):
  ZT[d, n]   = sum_m seq_s[m, d] * adjT[m, n]   (fp16, 32 accumulating
                                                 matmuls per psum bank,
                                                 split into two m-halves so
                                                 half 1's epilogue overlaps
                                                 half 2's stream)
  aggT[h, n] = sum_d (W/256)[d, h] * ZT[d, n]   (fp16 via a cast in the
                                                 PSUM->SBUF copy)
  hT         = relu(aggT + b)       (fp16 out; branch 0 on scalar with a
                                     free-axis accum -> community sums,
                                     branch 1 on vector as add+max)
  c          = sigmoid(sum / 1024)               [128, 1] fp16
  cw         = Wb @ c     (lhsT = Wb^T from host) [128, 1] fp16
  sc_s[n]    = sum_h hT_s[h, n] * cw[h] + bb     [1, 1024] per branch

Schedule notes (from trace analysis of the 93 us / 89 us baselines):
  - DMA: per-group [seq chunk][adj group] pairs alternate between the two
    HWDGE queues (sync + scalar); groups start at 1 m-tile and grow to 8
    because the queues ramp to their ~430 GB/s aggregate over the first
    ~6 us - a big first group just delays the first matmul.
  - PE: ~8 dummy matmuls on a memset scratch tile run during the initial
    DMA wait so the HAM throttle (cold 1.2 GHz -> warm 2.4 GHz, ~3.4 us
    activity window) is warm when real data arrives.
  - Scalar: a dummy 1-element sigmoid is the first activation so the
    activation-table pass loads the sigmoid table once at t~7 us instead
    of on the epilogue critical path.
  - The final m-tile group is emitted bank-major (b0c0|b0c1|b1c0|b1c1
    blocks) so branch 0's PSUM banks close ~4 us before the stream ends
    and the relu->sigmoid->cw chain overlaps branch 1's last matmuls.
"""

import numpy as np

import concourse.bass as bass
import concourse.tile as tile
from concourse import bacc, mybir
from concourse.bass_utils import run_bass_kernel_spmd

N = 8192          # nodes
D = 128           # input feature dim
H = 128           # hidden dim
NC = 8            # communities / cores
CS = N // NC      # community size (nodes per core)
MT = N // 128     # number of 128-row m-tiles (64)
CHUNK = 512       # matmul moving free dim (psum bank width in fp32)
NCH = CS // CHUNK # n-chunks per core (2)
HALF = MT // 2    # m-tiles per psum accumulation half (32)

F32 = mybir.dt.float32
F16 = mybir.dt.float16
ADJ_SCALE = 256.0  # keeps fp16(adj*scale) in the normal range; undone via W/256

# m-tile counts per DMA group; groups alternate sync/scalar queues and
# grow with the DMA-queue ramp. The last group is emitted bank-major.
GROUPS = [2, 2, 4, 4, 6, 6, 6, 6, 8, 8, 6, 6]
assert sum(GROUPS) == MT
N_DUMMY_MM = 8     # PE warmup matmuls during the initial DMA wait


def _build_module() -> bass.Bass:
    nc = bacc.Bacc()

    blk = nc.declare_dram_parameter("blk", [128, MT, 2 * D + CS], F16, isOutput=False)
    w = nc.declare_dram_parameter("w", [D, H], F16, isOutput=False)
    wbt = nc.declare_dram_parameter("wbt", [H, H], F16, isOutput=False)
    bvec = nc.declare_dram_parameter("bvec", [H, 1], F32, isOutput=False)
    bbvec = nc.declare_dram_parameter("bbvec", [1, 1], F32, isOutput=False)
    out = nc.declare_dram_parameter("out", [2, CS], F32, isOutput=True)

    with tile.TileContext(nc) as tc:
        _emit(tc, blk, w, wbt, bvec, bbvec, out)
    nc.finalize()
    return nc


def _emit(tc, blk, w, wbt, bvec, bbvec, out):
    nc = tc.nc
    Act = mybir.ActivationFunctionType
    Alu = mybir.AluOpType
    cw_box: list = []
    with (
        tc.tile_pool(name="singles", bufs=1) as singles,
        tc.tile_pool(name="adj_sync", bufs=4) as adj_sync,
        tc.tile_pool(name="adj_scal", bufs=4) as adj_scal,
        tc.tile_pool(name="misc", bufs=1) as misc,
        tc.tile_pool(name="psum", bufs=1, space="PSUM") as psum,
    ):
        # --- warmup: PE HAM ramp + activation-table preload ------------
        scratch = singles.tile([128, CHUNK], F16)
        nc.vector.memset(scratch, 1.0)
        sig_dummy = misc.tile([1, 1], F32)
        nc.scalar.activation(out=sig_dummy, in_=scratch[:1, :1], func=Act.Sigmoid)

        # params on the gpsimd (SWDGE) queue so they never contend with the
        # adj/seq HWDGE streams.
        w_sb = singles.tile([D, H], F16)
        nc.gpsimd.dma_start(out=w_sb, in_=w[:])
        wbt_sb = singles.tile([H, H], F16)
        nc.gpsimd.dma_start(out=wbt_sb, in_=wbt[:])
        b_sb = singles.tile([H, 1], F32)
        nc.gpsimd.dma_start(out=b_sb, in_=bvec[:])
        bb_sb = singles.tile([1, 1], F32)
        nc.gpsimd.dma_start(out=bb_sb, in_=bbvec[:])

        # Z accumulators split by m-half: first half banks 0-3, second 4-7.
        z_half = [
            [
                [psum.tile([128, CHUNK], F32, name=f"z_ps_{h}_{s}_{c}") for c in range(NCH)]
                for s in range(2)
            ]
            for h in range(2)
        ]
        zt_sb = [
            [
                [misc.tile([128, CHUNK], F16, name=f"zt_sb_{h}_{s}_{c}") for c in range(NCH)]
                for s in range(2)
            ]
            for h in range(2)
        ]
        h_sb = [
            [misc.tile([128, CHUNK], F16, name=f"h_sb_{s}_{c}") for c in range(NCH)]
            for s in range(2)
        ]
        csum = [misc.tile([H, 1], F32, name=f"csum_{c}") for c in range(NCH)]

        # PE warmup matmuls into a bank whose first real write (start=True)
        # is 30+ us away; they only read the memset scratch tile.
        for _ in range(N_DUMMY_MM):
            nc.tensor.matmul(
                z_half[1][1][1], scratch[:, :128], scratch, start=True, stop=True
            )

        def mm(t, s, c, blk_sb, u):
            nc.tensor.matmul(
                z_half[0 if t < HALF else 1][s][c],
                blk_sb[:, u, s * D : (s + 1) * D],
                blk_sb[:, u, 2 * D + c * CHUNK : 2 * D + (c + 1) * CHUNK],
                start=(t % HALF == 0),
                stop=(t % HALF == HALF - 1),
            )

        # --- main stream: per-group [seq chunk][adj group] on alternating
        # HWDGE queues, then the group's matmuls -------------------------
        t0 = 0
        for gi, gn in enumerate(GROUPS):
            q = nc.sync if gi % 2 == 0 else nc.scalar
            pool = adj_sync if gi % 2 == 0 else adj_scal
            blk_sb = pool.tile(
                [128, gn, 2 * D + CS], F16,
                name=f"blk_sb_{gi % 2}", tag=f"blk_{gi % 2}", bufs=4,
            )
            q.dma_start(out=blk_sb, in_=blk[:, t0 : t0 + gn, :])
            last = gi == len(GROUPS) - 1
            if not last:
                for u in range(gn):
                    t = t0 + u
                    for s in range(2):
                        for c in range(NCH):
                            mm(t, s, c, blk_sb, u)
            else:
                # Bank-major: each (s,c) accumulator closes as early as
                # possible; b0's epilogue overlaps b1's matmul blocks.
                for s in range(2):
                    for c in range(NCH):
                        if s == 1 and c == 1:
                            break
                        for u in range(gn):
                            mm(t0 + u, s, c, blk_sb, u)
                # b0 epilogue: casts, W-contraction close, relu+accum.
                # Emitted right after b0's banks close so the whole chain
                # runs while the PE streams b1's last matmuls.
                nc.vector.tensor_copy(out=zt_sb[1][0][0], in_=z_half[1][0][0])
                nc.scalar.activation(
                    out=zt_sb[1][0][1], in_=z_half[1][0][1], func=Act.Copy
                )
                for c in range(NCH):
                    nc.tensor.matmul(
                        z_half[0][0][c], w_sb, zt_sb[1][0][c], start=False, stop=True
                    )
                    nc.scalar.activation(
                        out=h_sb[0][c],
                        in_=z_half[0][0][c],
                        func=Act.Relu,
                        bias=b_sb,
                        accum_out=csum[c],
                    )
                for u in range(gn // 2):
                    mm(t0 + u, 1, 1, blk_sb, u)
                # cw chain slots into the middle of the b1c1 block so the
                # PE picks it up the moment sigmoid lands.
                csum_tot = misc.tile([H, 1], F32)
                nc.vector.tensor_add(out=csum_tot, in0=csum[0], in1=csum[1])
                c_sb = misc.tile([H, 1], F16)
                nc.scalar.activation(
                    out=c_sb, in_=csum_tot, func=Act.Sigmoid, scale=1.0 / CS
                )
                cw_ps = z_half[1][0][0]
                nc.tensor.matmul(cw_ps[:, :1], wbt_sb, c_sb, start=True, stop=True)
                cw_sb_t = misc.tile([H, 1], F16)
                nc.vector.tensor_copy(out=cw_sb_t, in_=cw_ps[:, :1])
                cw_box.append(cw_sb_t)
                for u in range(gn // 2, gn):
                    mm(t0 + u, 1, 1, blk_sb, u)
            t0 += gn
            if t0 - gn < HALF <= t0:
                # Mid-stream, off the critical path: all on vector so the
                # scalar engine's DMA trigger queue stays unblocked.
                for s in range(2):
                    for c in range(NCH):
                        nc.vector.tensor_copy(out=zt_sb[0][s][c], in_=z_half[0][s][c])
            if t0 - gn < HALF + 16 <= t0:
                # First-pass W-contraction into the (now free) half-1 banks.
                for s in range(2):
                    for c in range(NCH):
                        nc.tensor.matmul(
                            z_half[0][s][c], w_sb, zt_sb[0][s][c], start=True, stop=False
                        )

        # --- tail: b1 relu / scores / bias / store ----------------------
        cw_sb = cw_box[0]
        # b1 half-2 casts: c0 on vector, c1 on scalar (both engines work
        # the moment their bank closes).
        nc.vector.tensor_copy(out=zt_sb[1][1][0], in_=z_half[1][1][0])
        nc.scalar.activation(out=zt_sb[1][1][1], in_=z_half[1][1][1], func=Act.Copy)
        nc.tensor.matmul(z_half[0][1][0], w_sb, zt_sb[1][1][0], start=False, stop=True)
        # b1 relus: c0 on vector (add bias, clamp at 0), fp16 out.
        nc.vector.tensor_scalar(
            out=h_sb[1][0], in0=z_half[0][1][0],
            scalar1=b_sb, scalar2=0.0, op0=Alu.add, op1=Alu.max,
        )
        sc_banks = [
            [z_half[1][0][1], z_half[1][1][0]],
            [z_half[1][1][1], z_half[0][0][0]],
        ]
        out_sb = misc.tile([1, 2, CS], F32)

        def sc_mm(s, c):
            nc.tensor.matmul(
                sc_banks[s][c][:1, :], cw_sb, h_sb[s][c], start=True, stop=True
            )

        sc_mm(0, 0)
        sc_mm(0, 1)
        sc_mm(1, 0)
        nc.tensor.matmul(z_half[0][1][1], w_sb, zt_sb[1][1][1], start=False, stop=True)
        nc.scalar.activation(
            out=h_sb[1][1], in_=z_half[0][1][1], func=Act.Relu, bias=b_sb
        )
        sc_mm(1, 1)

        # +bb and PSUM->SBUF staging, split across vector and scalar.
        def add_v(s, c):
            nc.vector.tensor_scalar_add(
                out=out_sb[:, s, c * CHUNK : (c + 1) * CHUNK],
                in0=sc_banks[s][c][:1, :], scalar1=bb_sb,
            )

        def add_s(s, c):
            nc.scalar.activation(
                out=out_sb[:, s, c * CHUNK : (c + 1) * CHUNK],
                in_=sc_banks[s][c][:1, :], func=Act.Identity, bias=bb_sb,
            )

        add_v(0, 0)
        add_s(0, 1)
        add_v(1, 0)
        add_s(1, 1)
        # single HWDGE store of both branches' scores
        nc.sync.dma_start(out=out[:, :].unsqueeze(0), in_=out_sb)



def get_module() -> bass.Bass:
    if not _MODULE_CACHE:
        _MODULE_CACHE.append(_build_module())
    return _MODULE_CACHE[0]


_MODULE_CACHE: list = []


def shard_inputs(inputs: dict) -> list[dict]:
    """Full inputs -> per-core input maps (row-block sharding of adjT)."""
    def tile_seq(s):
        s16 = np.asarray(s, np.float32)[0].astype(np.float16)  # [N, D]
        return s16.reshape(MT, 128, D).transpose(1, 0, 2)

    # both branches' seq tiles, packed ahead of each adj tile so one DMA
    # per group delivers the matmul's stationary and moving operands
    s1 = tile_seq(inputs["seq1"])  # [128, MT, D]
    s2 = tile_seq(inputs["seq2"])
    adj16 = (np.asarray(inputs["adj"], np.float32)[0] * ADJ_SCALE).astype(np.float16)
    # fold the adj prescale's inverse into W so the relu needs no scale
    w = np.ascontiguousarray(
        (np.asarray(inputs["W"], np.float32) / ADJ_SCALE).astype(np.float16)
    )
    wbt = np.ascontiguousarray(np.asarray(inputs["Wb"], np.float32).T.astype(np.float16))
    bvec = np.asarray(inputs["b"], np.float32).reshape(H, 1).copy()
    bbvec = np.asarray(inputs["bb"], np.float32).reshape(1, 1).copy()

    in_maps = []
    for k in range(NC):
        in_maps.append(
            {
                "blk": np.ascontiguousarray(
                    np.concatenate(
                        [
                            s1,
                            s2,
                            adj16[k * CS : (k + 1) * CS, :].T
                            .reshape(MT, 128, CS).transpose(1, 0, 2),
                        ],
                        axis=2,
                    )
                ),
                "w": w,
                "wbt": wbt,
                "bvec": bvec,
                "bbvec": bbvec,
            }
        )
    return in_maps


def gather_output(core_outs: list[np.ndarray], cc_label: np.ndarray) -> np.ndarray:
    """Per-core [2, CS] score blocks -> full [1, 2N] output.

    Scatter through cc_label mirrors the reference's .at[flat].set: entry
    (community k, position j) is the score of node cc_label[k, j].
    """
    sc1 = np.concatenate([o[0] for o in core_outs]).astype(np.float32)
    sc2 = np.concatenate([o[1] for o in core_outs]).astype(np.float32)
    flat = np.asarray(cc_label).reshape(-1)
    ret1 = np.zeros(N, np.float32)
    ret2 = np.zeros(N, np.float32)
    ret1[flat] = sc1
    ret2[flat] = sc2
    return np.concatenate([ret1, ret2])[None, :]


def kernel(**inputs) -> np.ndarray:
    nc = get_module()
    in_maps = shard_inputs(inputs)
    res = run_bass_kernel_spmd(nc, in_maps, core_ids=list(range(NC)))
    core_outs = [res.results[k]["out"] for k in range(NC)]
    return gather_output(core_outs, inputs["cc_label"])


if __name__ == "__main__":
    nc = get_module()
    print("module built ok")


# revision 8
# speedup vs baseline: 1.1026x; 1.1026x over previous
"""DGI (Deep Graph Infomax) forward kernel for 8 TRN2 NeuronCores.

Problem (all shapes hardcoded):
  seq1, seq2: [1, 8192, 128] f32   node features
  adj:        [1, 8192, 8192] f32  dense adjacency
  cc_label:   [8, 1024] i32        community partition (arange layout)
  W: [128,128], b: [128], Wb: [128,128], bb: [] f32
  out:        [1, 16384] f32       = concat(ret1, ret2)

Math per GCN branch: h = relu(adj @ (seq @ W) + b). We reassociate to
(adj @ seq) @ W so the big contraction uses natural-layout seq tiles as
the stationary operand and a host-transposed adj block as the moving
operand; everything then lives in "transposed" space (features on
partitions), where the community mean is a free-axis reduction and the
bilinear scores are a 1-column matmul.

Sharding: core k owns nodes [1024k, 1024k+1024) == community k (cc_label
is arange). Each core reads its adjT column block (16 MB fp16), the full
seqs (4 MB fp16, replicated), computes its 1024 scores per branch. No
collectives.

Per-core device program (everything on the PE in fp16 so nothing runs at
the fp32 quarter rate; adj is pre-scaled by 256 on the host to sit in
fp16's normal range; the inverse scale is folded into W on the host):
  ZT[d, n]   = sum_m seq_s[m, d] * adjT[m, n]   (fp16, 32 accumulating
                                                 matmuls per psum bank,
                                                 split into two m-halves so
                                                 half 1's epilogue overlaps
                                                 half 2's stream)
  aggT[h, n] = sum_d (W/256)[d, h] * ZT[d, n]   (fp16 via a cast in the
                                                 PSUM->SBUF copy)
  hT         = relu(aggT + b)       (fp16 out; branch 0 on scalar with a
                                     free-axis accum -> community sums,
                                     branch 1 on vector as add+max)
  c          = sigmoid(sum / 1024)               [128, 1] fp16
  cw         = Wb @ c     (lhsT = Wb^T from host) [128, 1] fp16
  sc_s[n]    = sum_h hT_s[h, n] * cw[h] + bb     [1, 1024] per branch

Schedule notes (from trace analysis of the 93 us / 89 us baselines):
  - DMA: per-group [seq chunk][adj group] pairs alternate between the two
    HWDGE queues (sync + scalar); groups start at 1 m-tile and grow to 8
    because the queues ramp to their ~430 GB/s aggregate over the first
    ~6 us - a big first group just delays the first matmul.
  - PE: ~8 dummy matmuls on a memset scratch tile run during the initial
    DMA wait so the HAM throttle (cold 1.2 GHz -> warm 2.4 GHz, ~3.4 us
    activity window) is warm when real data arrives.
  - Scalar: a dummy 1-element sigmoid is the first activation so the
    activation-table pass loads the sigmoid table once at t~7 us instead
    of on the epilogue critical path.
  - The final m-tile group is emitted bank-major (b0c0|b0c1|b1c0|b1c1
    blocks) so branch 0's PSUM banks close ~4 us before the stream ends
    and the relu->sigmoid->cw chain overlaps branch 1's last matmuls.
"""

import numpy as np

import concourse.bass as bass
import concourse.tile as tile
from concourse import bacc, mybir
from concourse.bass_utils import run_bass_kernel_spmd

N = 8192          # nodes
D = 128           # input feature dim
H = 128           # hidden dim
NC = 8            # communities / cores
CS = N // NC      # community size (nodes per core)
MT = N // 128     # number of 128-row m-tiles (64)
CHUNK = 512       # matmul moving free dim (psum bank width in fp32)
NCH = CS // CHUNK # n-chunks per core (2)
HALF = MT // 2    # m-tiles per psum accumulation half (32)

F32 = mybir.dt.float32
F16 = mybir.dt.float16
ADJ_SCALE = 256.0  # keeps fp16(adj*scale) in the normal range; undone via W/256

# m-tile counts per adj DMA group, all on the sync HWDGE queue (few big
# uniform transfers measured fastest); seq chunks ride the scalar queue.
# The last group is emitted bank-major.
GROUPS = [2, 6, 8, 8, 8, 8, 8, 8, 8]
SEQ_CHUNKS = [8, 8, 16, 16, 16]
assert sum(GROUPS) == MT and sum(SEQ_CHUNKS) == MT
N_DUMMY_MM = 8     # PE warmup matmuls during the initial DMA wait


def _build_module() -> bass.Bass:
    nc = bacc.Bacc()

    adjt = nc.declare_dram_parameter("adjt", [128, MT, CS], F16, isOutput=False)
    seqs = nc.declare_dram_parameter("seqs", [128, MT, 2, D], F16, isOutput=False)
    w = nc.declare_dram_parameter("w", [D, H], F16, isOutput=False)
    wbt = nc.declare_dram_parameter("wbt", [H, H], F16, isOutput=False)
    bvec = nc.declare_dram_parameter("bvec", [H, 1], F32, isOutput=False)
    bbvec = nc.declare_dram_parameter("bbvec", [1, 1], F32, isOutput=False)
    out = nc.declare_dram_parameter("out", [2, CS], F32, isOutput=True)

    with tile.TileContext(nc) as tc:
        _emit(tc, adjt, seqs, w, wbt, bvec, bbvec, out)
    nc.finalize()
    return nc


def _emit(tc, adjt, seqs, w, wbt, bvec, bbvec, out):
    nc = tc.nc
    Act = mybir.ActivationFunctionType
    Alu = mybir.AluOpType
    cw_box: list = []
    with (
        tc.tile_pool(name="singles", bufs=1) as singles,
        tc.tile_pool(name="adj_sync", bufs=4) as adj_sync,
        tc.tile_pool(name="misc", bufs=1) as misc,
        tc.tile_pool(name="psum", bufs=1, space="PSUM") as psum,
    ):
        # --- warmup: PE HAM ramp + activation-table preload ------------
        scratch = singles.tile([128, CHUNK], F16)
        nc.vector.memset(scratch, 1.0)
        sig_dummy = misc.tile([1, 1], F32)
        nc.scalar.activation(out=sig_dummy, in_=scratch[:1, :1], func=Act.Sigmoid)

        # params on the gpsimd (SWDGE) queue so they never contend with the
        # adj/seq HWDGE streams.
        w_sb = singles.tile([D, H], F16)
        nc.gpsimd.dma_start(out=w_sb, in_=w[:])
        wbt_sb = singles.tile([H, H], F16)
        nc.gpsimd.dma_start(out=wbt_sb, in_=wbt[:])
        b_sb = singles.tile([H, 1], F32)
        nc.gpsimd.dma_start(out=b_sb, in_=bvec[:])
        bb_sb = singles.tile([1, 1], F32)
        nc.gpsimd.dma_start(out=bb_sb, in_=bbvec[:])

        # Z accumulators split by m-half: first half banks 0-3, second 4-7.
        z_half = [
            [
                [psum.tile([128, CHUNK], F32, name=f"z_ps_{h}_{s}_{c}") for c in range(NCH)]
                for s in range(2)
            ]
            for h in range(2)
        ]
        zt_sb = [
            [
                [misc.tile([128, CHUNK], F16, name=f"zt_sb_{h}_{s}_{c}") for c in range(NCH)]
                for s in range(2)
            ]
            for h in range(2)
        ]
        h_sb = [
            [misc.tile([128, CHUNK], F16, name=f"h_sb_{s}_{c}") for c in range(NCH)]
            for s in range(2)
        ]
        csum = [misc.tile([H, 1], F32, name=f"csum_{c}") for c in range(NCH)]

        seq_sb = singles.tile([128, MT, 2, D], F16)
        pos = 0
        for n in SEQ_CHUNKS:
            nc.scalar.dma_start(
                out=seq_sb[:, pos : pos + n, :, :], in_=seqs[:, pos : pos + n, :, :]
            )
            pos += n

        # PE warmup matmuls into a bank whose first real write (start=True)
        # is 30+ us away; they only read the memset scratch tile.
        for _ in range(N_DUMMY_MM):
            nc.tensor.matmul(
                z_half[1][1][1], scratch[:, :128], scratch, start=True, stop=True
            )

        def mm(t, s, c, adj_sb, u):
            nc.tensor.matmul(
                z_half[0 if t < HALF else 1][s][c],
                seq_sb[:, t, s, :],
                adj_sb[:, u, c * CHUNK : (c + 1) * CHUNK],
                start=(t % HALF == 0),
                stop=(t % HALF == HALF - 1),
            )

        # --- main stream: per-group [seq chunk][adj group] on alternating
        # HWDGE queues, then the group's matmuls -------------------------
        t0 = 0
        for gi, gn in enumerate(GROUPS):
            adj_sb = adj_sync.tile(
                [128, gn, CS], F16, name="adj_sb", tag="adj_sb", bufs=4
            )
            nc.sync.dma_start(out=adj_sb, in_=adjt[:, t0 : t0 + gn, :])
            last = gi == len(GROUPS) - 1
            if not last:
                for u in range(gn):
                    t = t0 + u
                    for s in range(2):
                        for c in range(NCH):
                            mm(t, s, c, adj_sb, u)
            else:
                # Bank-major: each (s,c) accumulator closes as early as
                # possible; b0's epilogue overlaps b1's matmul blocks.
                for s in range(2):
                    for c in range(NCH):
                        if s == 1 and c == 1:
                            break
                        for u in range(gn):
                            mm(t0 + u, s, c, adj_sb, u)
                # b0 epilogue: casts, W-contraction close, relu+accum.
                # Emitted right after b0's banks close so the whole chain
                # runs while the PE streams b1's last matmuls.
                nc.vector.tensor_copy(out=zt_sb[1][0][0], in_=z_half[1][0][0])
                nc.scalar.activation(
                    out=zt_sb[1][0][1], in_=z_half[1][0][1], func=Act.Copy
                )
                for c in range(NCH):
                    nc.tensor.matmul(
                        z_half[0][0][c], w_sb, zt_sb[1][0][c], start=False, stop=True
                    )
                    nc.scalar.activation(
                        out=h_sb[0][c],
                        in_=z_half[0][0][c],
                        func=Act.Relu,
                        bias=b_sb,
                        accum_out=csum[c],
                    )
                for u in range(gn // 2):
                    mm(t0 + u, 1, 1, adj_sb, u)
                # cw chain slots into the middle of the b1c1 block so the
                # PE picks it up the moment sigmoid lands.
                csum_tot = misc.tile([H, 1], F32)
                nc.vector.tensor_add(out=csum_tot, in0=csum[0], in1=csum[1])
                c_sb = misc.tile([H, 1], F16)
                nc.scalar.activation(
                    out=c_sb, in_=csum_tot, func=Act.Sigmoid, scale=1.0 / CS
                )
                cw_ps = z_half[1][0][0]
                nc.tensor.matmul(cw_ps[:, :1], wbt_sb, c_sb, start=True, stop=True)
                cw_sb_t = misc.tile([H, 1], F16)
                nc.vector.tensor_copy(out=cw_sb_t, in_=cw_ps[:, :1])
                cw_box.append(cw_sb_t)
                for u in range(gn // 2, gn):
                    mm(t0 + u, 1, 1, adj_sb, u)
            t0 += gn
            if t0 - gn < HALF <= t0:
                # Mid-stream, off the critical path: all on vector so the
                # scalar engine's DMA trigger queue stays unblocked.
                for s in range(2):
                    for c in range(NCH):
                        nc.vector.tensor_copy(out=zt_sb[0][s][c], in_=z_half[0][s][c])
            if t0 - gn < HALF + 16 <= t0:
                # First-pass W-contraction into the (now free) half-1 banks.
                for s in range(2):
                    for c in range(NCH):
                        nc.tensor.matmul(
                            z_half[0][s][c], w_sb, zt_sb[0][s][c], start=True, stop=False
                        )

        # --- tail: b1 relu / scores / bias / store ----------------------
        cw_sb = cw_box[0]
        # b1 half-2 casts: c0 on vector, c1 on scalar (both engines work
        # the moment their bank closes).
        nc.vector.tensor_copy(out=zt_sb[1][1][0], in_=z_half[1][1][0])
        nc.scalar.activation(out=zt_sb[1][1][1], in_=z_half[1][1][1], func=Act.Copy)
        nc.tensor.matmul(z_half[0][1][0], w_sb, zt_sb[1][1][0], start=False, stop=True)
        # b1 relus: c0 on vector (add bias, clamp at 0), fp16 out.
        nc.vector.tensor_scalar(
            out=h_sb[1][0], in0=z_half[0][1][0],
            scalar1=b_sb, scalar2=0.0, op0=Alu.add, op1=Alu.max,
        )
        sc_banks = [
            [z_half[1][0][1], z_half[1][1][0]],
            [z_half[1][1][1], z_half[0][0][0]],
        ]
        out_sb = misc.tile([1, 2, CS], F32)

        def sc_mm(s, c):
            nc.tensor.matmul(
                sc_banks[s][c][:1, :], cw_sb, h_sb[s][c], start=True, stop=True
            )

        sc_mm(0, 0)
        sc_mm(0, 1)
        sc_mm(1, 0)
        nc.tensor.matmul(z_half[0][1][1], w_sb, zt_sb[1][1][1], start=False, stop=True)
        nc.scalar.activation(
            out=h_sb[1][1], in_=z_half[0][1][1], func=Act.Relu, bias=b_sb
        )
        sc_mm(1, 1)

        # +bb and PSUM->SBUF staging, split across vector and scalar.
        def add_v(s, c):
            nc.vector.tensor_scalar_add(
                out=out_sb[:, s, c * CHUNK : (c + 1) * CHUNK],
                in0=sc_banks[s][c][:1, :], scalar1=bb_sb,
            )

        def add_s(s, c):
            nc.scalar.activation(
                out=out_sb[:, s, c * CHUNK : (c + 1) * CHUNK],
                in_=sc_banks[s][c][:1, :], func=Act.Identity, bias=bb_sb,
            )

        add_v(0, 0)
        add_s(0, 1)
        add_v(1, 0)
        add_s(1, 1)
        # single HWDGE store of both branches' scores
        nc.sync.dma_start(out=out[:, :].unsqueeze(0), in_=out_sb)



def get_module() -> bass.Bass:
    if not _MODULE_CACHE:
        _MODULE_CACHE.append(_build_module())
    return _MODULE_CACHE[0]


_MODULE_CACHE: list = []


def shard_inputs(inputs: dict) -> list[dict]:
    """Full inputs -> per-core input maps (row-block sharding of adjT)."""
    def tile_seq(s):
        s16 = np.asarray(s, np.float32)[0].astype(np.float16)  # [N, D]
        return s16.reshape(MT, 128, D).transpose(1, 0, 2)

    # interleave both branches so one seq chunk feeds both: [128, MT, 2, D]
    seqs = np.ascontiguousarray(
        np.stack([tile_seq(inputs["seq1"]), tile_seq(inputs["seq2"])], axis=2)
    )
    adj16 = (np.asarray(inputs["adj"], np.float32)[0] * ADJ_SCALE).astype(np.float16)
    # fold the adj prescale's inverse into W so the relu needs no scale
    w = np.ascontiguousarray(
        (np.asarray(inputs["W"], np.float32) / ADJ_SCALE).astype(np.float16)
    )
    wbt = np.ascontiguousarray(np.asarray(inputs["Wb"], np.float32).T.astype(np.float16))
    bvec = np.asarray(inputs["b"], np.float32).reshape(H, 1).copy()
    bbvec = np.asarray(inputs["bb"], np.float32).reshape(1, 1).copy()

    in_maps = []
    for k in range(NC):
        in_maps.append(
            {
                "adjt": np.ascontiguousarray(
                    adj16[k * CS : (k + 1) * CS, :].T.reshape(MT, 128, CS).transpose(1, 0, 2)
                ),
                "seqs": seqs,
                "w": w,
                "wbt": wbt,
                "bvec": bvec,
                "bbvec": bbvec,
            }
        )
    return in_maps


def gather_output(core_outs: list[np.ndarray], cc_label: np.ndarray) -> np.ndarray:
    """Per-core [2, CS] score blocks -> full [1, 2N] output.

    Scatter through cc_label mirrors the reference's .at[flat].set: entry
    (community k, position j) is the score of node cc_label[k, j].
    """
    sc1 = np.concatenate([o[0] for o in core_outs]).astype(np.float32)
    sc2 = np.concatenate([o[1] for o in core_outs]).astype(np.float32)
    flat = np.asarray(cc_label).reshape(-1)
    ret1 = np.zeros(N, np.float32)
    ret2 = np.zeros(N, np.float32)
    ret1[flat] = sc1
    ret2[flat] = sc2
    return np.concatenate([ret1, ret2])[None, :]


def kernel(**inputs) -> np.ndarray:
    nc = get_module()
    in_maps = shard_inputs(inputs)
    res = run_bass_kernel_spmd(nc, in_maps, core_ids=list(range(NC)))
    core_outs = [res.results[k]["out"] for k in range(NC)]
    return gather_output(core_outs, inputs["cc_label"])


if __name__ == "__main__":
    nc = get_module()
    print("module built ok")
